# revision 14
# baseline (speedup 1.0000x reference)
"""CTAN (gnn_message_passing) Trainium2 kernel — 8 NeuronCores, edge-parallel.

V3 design (vs baseline): gather x[src] (bf16, 256B rows) instead of k|v
(512B); k/v applied implicitly via per-window projections qk = wk^T q',
qe = we^T q' so the score is x[src]*qk + attr*qe (bk cancels in softmax,
bv folds into abias via sum(a)=1). Host precomputes attr = [msg | cos(te)]
in both orientations plus a {0,-1024} onehot mask table, so iterations are
uniform (no Sin/Abs ACT table thrash) and DVE work per chunk is two ops.
Scores, scatter, and denominators all run as PE matmuls; exp runs
full-matrix on ACT. Node phase is interleaved into edge-phase window
epilogues; the only iteration barrier is the AllGather of x (bf16, Shared
output).
"""
import sys
import os
import math
import numpy as np

sys.path.insert(0, "/opt/trn_rl_repo")

MEM = 128
NODE = 128
EDGE = 72
TIME = 56
ITERS = 3
EPS = 0.1
GAMMA = 0.1
NCORES = 8
P = 128
GWIN = 2          # windows per edge-phase group
LO_LIMIT = int(os.environ.get("KERNEL_LO_LIMIT", "32768"))  # int16 gather limit
BIG = 1024.0      # softmax mask offset
MAXB = 8          # max chunks per dma_gather call
INV_SQRT_D = 1.0 / math.sqrt(MEM)


def _bf16(a):
    import ml_dtypes
    return np.asarray(a, dtype=np.float32).astype(ml_dtypes.bfloat16)


def _wrap16(a):
    """int16 index list -> [128, n/16] dma_gather layout."""
    a = np.asarray(a, dtype=np.int16)
    assert len(a) % 16 == 0
    return np.tile(a.reshape(-1, 16).T, (8, 1)).astype(np.int16)


def _host_prep(n_id, edge_index, t, msg, last_update, time_w, time_b):
    N = n_id.shape[0]
    E = edge_index.shape[1]
    src = np.asarray(edge_index[0], dtype=np.int64)
    dst = np.asarray(edge_index[1], dtype=np.int64)

    deg = np.bincount(dst, minlength=N)
    cum = np.cumsum(deg)
    # contiguous node ranges with ~equal edge counts
    bounds = [0]
    for c in range(1, NCORES):
        bounds.append(int(np.searchsorted(cum, E * c / NCORES)))
    bounds.append(N)
    node_core = np.zeros(N, dtype=np.int64)
    for c in range(NCORES):
        node_core[bounds[c]:bounds[c + 1]] = c
    ncnt = [bounds[c + 1] - bounds[c] for c in range(NCORES)]
    NW = max(1, math.ceil(max(ncnt) / P))
    NW = math.ceil(NW / GWIN) * GWIN
    NSH = NW * P
    NFULL = NCORES * NSH
    assert NFULL - LO_LIMIT < LO_LIMIT, "hi table exceeds int16 range"

    # per-core node->window assignment: greedy bin-pack by in-degree so
    # window edge counts are near-uniform (minimizes chunk padding)
    local_of = np.full(N, -1, dtype=np.int64)
    nid_own = np.zeros((NCORES, NSH), dtype=np.int32)
    for c in range(NCORES):
        nodes = np.arange(bounds[c], bounds[c + 1])
        order = nodes[np.argsort(-deg[nodes], kind="stable")]
        wload = np.zeros(NW, dtype=np.int64)
        wslots = np.zeros(NW, dtype=np.int64)
        for n in order:
            cand = np.nonzero(wslots < P)[0]
            w = int(cand[np.argmin(wload[cand])])
            local_of[n] = w * P + wslots[w]
            nid_own[c, w * P + wslots[w]] = n_id[n]
            wslots[w] += 1
            wload[w] += deg[n]
    glob_row = node_core * NSH + local_of  # x_full row of each original node

    # edge features: attr = [msg | cos(|lu[nid[src]] - t| * tw + tb)]
    lu = np.asarray(last_update, dtype=np.int64)
    nid64 = np.asarray(n_id, dtype=np.int64)
    rel = np.abs(lu[nid64[src]] - np.asarray(t, dtype=np.int64)).astype(np.float32)
    te = np.cos(rel[:, None] * np.asarray(time_w, np.float32)[None, :]
                + np.asarray(time_b, np.float32)[None, :]).astype(np.float32)
    attr = np.concatenate([np.asarray(msg, np.float32), te], axis=1)  # [E,128]
    assert attr.shape[1] == P

    # edges per core, windowed, lo/hi split on src row
    e_core = node_core[dst]
    ld_all = local_of[dst]          # 0..NSH-1 within dst core
    e_win = ld_all // P
    srcrow = glob_row[src]
    is_lo = srcrow < LO_LIMIT

    KL = 0
    KH = 0
    per_core_win_edges = []
    for c in range(NCORES):
        m = e_core == c
        wins = []
        for w in range(NW):
            mw = m & (e_win == w)
            elo = np.nonzero(mw & is_lo)[0]
            ehi = np.nonzero(mw & ~is_lo)[0]
            wins.append((elo, ehi))
            KL = max(KL, math.ceil(len(elo) / P))
            KH = max(KH, math.ceil(len(ehi) / P))
        per_core_win_edges.append(wins)
    NCH_W = KL + KH
    NCHUNK = NW * NCH_W
    EP = NCHUNK * P                # padded edge slots per core
    ELO = NW * KL * P
    EHI = NW * KH * P

    cores = []
    for c in range(NCORES):
        attrT = np.zeros((P, NCHUNK * P), dtype=np.float32)   # [fa, slot]
        attrN = np.zeros((P, NCHUNK * P), dtype=np.float32)   # [p, ch*128+fa]
        Mt = np.full((P, NCHUNK * P), -BIG, dtype=np.float32)  # [p, ch*128+d]
        xlo = np.zeros(max(ELO, 16), dtype=np.int16)
        xhi = np.zeros(max(EHI, 16), dtype=np.int16)
        for w in range(NW):
            elo, ehi = per_core_win_edges[c][w]
            for which, elist, K, base_k, xarr, kbase in (
                (0, elo, KL, 0, xlo, w * KL * P),
                (1, ehi, KH, KL, xhi, w * KH * P),
            ):
                if K == 0 or len(elist) == 0:
                    continue
                n = len(elist)
                ch0 = w * NCH_W + base_k      # first chunk index
                pos = np.arange(n)
                chs = ch0 + pos // P          # chunk of each edge
                ps_ = pos % P                 # slot within chunk
                attrT[:, chs * P + ps_] = attr[elist].T
                attrN[ps_[:, None], (chs * P)[:, None] + np.arange(P)[None, :]] = attr[elist]
                Mt[ps_, chs * P + (ld_all[elist] % P)] = 0.0
                rows = srcrow[elist] - (LO_LIMIT if which else 0)
                xarr[kbase:kbase + n] = rows.astype(np.int16)
        cores.append(dict(
            attrT=_bf16(attrT), attrN=_bf16(attrN), Mt=_bf16(Mt),
            xlo=_wrap16(xlo), xhi=_wrap16(xhi),
            nid=nid_own[c].reshape(NSH, 1),
        ))

    meta = dict(N=N, E=E, NSH=NSH, NW=NW, KL=KL, KH=KH, NCH_W=NCH_W,
                NCHUNK=NCHUNK, EP=EP, ELO=max(ELO, 16), EHI=max(EHI, 16),
                bounds=bounds, local_of=local_of)
    return cores, meta


def _build(meta, num_nodes):
    import concourse.bacc as bacc
    import concourse.bass as bass
    import concourse.mybir as mybir
    import concourse.tile as tile
    from concourse.masks import make_identity

    dt = mybir.dt
    Alu = mybir.AluOpType
    Act = mybir.ActivationFunctionType

    NSH, NW, KL, KH, NCH_W, NCHUNK = (meta[k] for k in
                                      ("NSH", "NW", "KL", "KH", "NCH_W",
                                       "NCHUNK"))
    ELO, EHI = meta["ELO"], meta["EHI"]
    NFULL = NCORES * NSH
    NGRP = NW // GWIN
    GN = GWIN * NCH_W      # chunks per group
    GL = GWIN * KL         # lo chunks per group
    GH = GWIN * KH

    nc = bacc.Bacc("TRN2", target_bir_lowering=False, debug=False,
                   num_devices=NCORES)

    def din(name, shape, dtype):
        return nc.dram_tensor(name, shape, dtype, kind="ExternalInput")

    t_mem = din("memory", [num_nodes, MEM], dt.float32)
    t_stat = din("static_node_features", [num_nodes, NODE], dt.float32)
    t_nid = din("nid", [NSH, 1], dt.int32)
    t_attrT = din("attrT", [P, NCHUNK * P], dt.bfloat16)
    t_attrN = din("attrN", [P, NCHUNK * P], dt.bfloat16)
    t_Mt = din("Mt", [P, NCHUNK * P], dt.bfloat16)
    t_xlo = din("xlo", [P, ELO // 16], dt.int16)
    t_xhi = din("xhi", [P, EHI // 16], dt.int16)
    # host-prepared weights
    t_encwT = din("enc_wT", [MEM + NODE, MEM], dt.float32)
    t_encb = din("encb_rep", [P, MEM], dt.float32)      # replicated rows
    t_wqT = din("wqT_isd", [MEM, MEM], dt.float32)      # wq.T * isd
    t_bq = din("bq_isd", [MEM, 1], dt.float32)          # column
    t_wk = din("wk", [MEM, MEM], dt.float32)            # natural
    t_we = din("we", [MEM, MEM], dt.float32)            # natural [fm, fa]
    t_wvT = din("wvT", [MEM, MEM], dt.float32)          # wv.T [fx, fm]
    t_weT = din("weT", [MEM, MEM], dt.float32)          # we.T [fa, fm]
    t_arhs = din("A_rhs", [MEM, MEM], dt.float32)       # A.T = aW.T-aW-gI
    t_ab = din("abias2_rep", [P, MEM], dt.float32)      # (abias+bv) rows
    t_out = nc.dram_tensor("out", [NSH, MEM], dt.float32, kind="ExternalOutput")

    attrT_r = t_attrT.ap().rearrange("p (c f) -> p c f", f=P)
    attrN_r = t_attrN.ap().rearrange("p (c f) -> p c f", f=P)
    Mt_r = t_Mt.ap().rearrange("p (c f) -> p c f", f=P)

    with tile.TileContext(nc) as tc:
        perm = tc.alloc_tile_pool(name="perm", bufs=1)
        sb = tc.alloc_tile_pool(name="sb", bufs=2)
        sb3 = tc.alloc_tile_pool(name="sb3", bufs=3)
        ps = tc.alloc_tile_pool(name="ps", bufs=2, space="PSUM")
        psw = tc.alloc_tile_pool(name="psw", bufs=1, space="PSUM")
        dram = tc.alloc_tile_pool(name="dram", bufs=1, space="DRAM")

        # ---------- persistent DRAM ----------
        x_own = dram.tile([NSH, MEM], dt.bfloat16)
        x_fulls = [dram.tile([NFULL, MEM], dt.bfloat16, addr_space="Shared",
                             name=f"x_full_{i}") for i in range(ITERS)]

        # ---------- persistent SBUF ----------
        x_sb = perm.tile([P, NW, MEM], dt.float32)
        xa_sb = perm.tile([P, NW, MEM], dt.float32)
        qkT_sb = perm.tile([P, NW, MEM], dt.bfloat16)
        qeT_sb = perm.tile([P, NW, MEM], dt.bfloat16)
        nid_sb = perm.tile([P, NW], dt.int32)
        ident = perm.tile([P, P], dt.float32)
        ident_bf = perm.tile([P, P], dt.bfloat16)
        ones_col = perm.tile([P, 1], dt.bfloat16)
        wq_sb = perm.tile([MEM, MEM], dt.bfloat16)
        wk_sb = perm.tile([MEM, MEM], dt.bfloat16)
        we_sb = perm.tile([MEM, MEM], dt.bfloat16)
        wvT_sb = perm.tile([MEM, MEM], dt.bfloat16)
        weT_sb = perm.tile([MEM, MEM], dt.bfloat16)
        arhs_sb = perm.tile([MEM, MEM], dt.bfloat16)
        bq_sb = perm.tile([MEM, 1], dt.float32)
        ab_sb = perm.tile([P, MEM], dt.float32)
        encb_sb = perm.tile([P, MEM], dt.float32)
        encwT_sb = perm.tile([P, 2, MEM], dt.float32)

        # ---------- startup constants ----------
        make_identity(nc, ident[:])
        nc.vector.tensor_copy(out=ident_bf[:], in_=ident[:])
        nc.vector.memset(ones_col[:], 1.0)
        for dst_t, src_t in ((wq_sb, t_wqT), (wk_sb, t_wk), (we_sb, t_we),
                             (wvT_sb, t_wvT), (weT_sb, t_weT),
                             (arhs_sb, t_arhs)):
            tmp = sb3.tile([MEM, MEM], dt.float32, tag="wload")
            nc.sync.dma_start(out=tmp[:], in_=src_t[:])
            nc.vector.tensor_copy(out=dst_t[:], in_=tmp[:])
        nc.sync.dma_start(out=bq_sb[:], in_=t_bq[:])
        nc.sync.dma_start(out=ab_sb[:], in_=t_ab[:])
        nc.sync.dma_start(out=encb_sb[:], in_=t_encb[:])
        nc.sync.dma_start(out=encwT_sb[:, 0, :], in_=t_encwT[0:P, :])
        nc.sync.dma_start(out=encwT_sb[:, 1, :], in_=t_encwT[P:2 * P, :])
        nc.sync.dma_start(out=nid_sb[:], in_=t_nid.ap().rearrange(
            "(c p) one -> p (c one)", p=P))

        x_own_r = x_own[:].rearrange("(c p) f -> p c f", p=P)

        def node_prep(w):
            """q' projections + xa + x bf16 writeback for window w."""
            tp = ps.tile([P, P], dt.float32, space="PSUM", tag="nps")
            nc.tensor.transpose(out=tp[:], in_=x_sb[:, w, :], identity=ident[:])
            xt = sb3.tile([P, P], dt.bfloat16, tag="xt")
            nc.vector.tensor_copy(out=xt[:], in_=tp[:])
            qps = ps.tile([P, P], dt.float32, space="PSUM", tag="nps")
            nc.tensor.matmul(out=qps[:], lhsT=wq_sb[:], rhs=xt[:],
                             start=True, stop=True)
            qT = sb3.tile([P, P], dt.bfloat16, tag="qT")
            nc.vector.tensor_tensor(out=qT[:], in0=qps[:],
                                    in1=bq_sb[:].to_broadcast([P, P]),
                                    op=Alu.add)
            qkp = ps.tile([P, P], dt.float32, space="PSUM", tag="nps")
            nc.tensor.matmul(out=qkp[:], lhsT=wk_sb[:], rhs=qT[:],
                             start=True, stop=True)
            nc.vector.tensor_copy(out=qkT_sb[:, w, :], in_=qkp[:])
            qep = ps.tile([P, P], dt.float32, space="PSUM", tag="nps")
            nc.tensor.matmul(out=qep[:], lhsT=we_sb[:], rhs=qT[:],
                             start=True, stop=True)
            nc.vector.tensor_copy(out=qeT_sb[:, w, :], in_=qep[:])
            xap = ps.tile([P, P], dt.float32, space="PSUM", tag="nps")
            nc.tensor.matmul(out=xap[:], lhsT=xt[:], rhs=arhs_sb[:],
                             start=True, stop=True)
            nc.vector.tensor_tensor(out=xa_sb[:, w, :], in0=xap[:],
                                    in1=ab_sb[:], op=Alu.add)
            xbf = sb3.tile([P, P], dt.bfloat16, tag="xbf")
            nc.vector.tensor_copy(out=xbf[:], in_=x_sb[:, w, :])
            nc.sync.dma_start(out=x_own_r[:, w, :], in_=xbf[:])

        # ---------- encoder: x = [memory|static][n_id] @ enc_w.T + enc_b ----
        for c in range(NW):
            memg = sb3.tile([P, MEM], dt.float32, tag="memg")
            statg = sb3.tile([P, NODE], dt.float32, tag="statg")
            nc.gpsimd.indirect_dma_start(
                out=memg[:], out_offset=None, in_=t_mem[:],
                in_offset=bass.IndirectOffsetOnAxis(ap=nid_sb[:, c:c + 1], axis=0))
            nc.gpsimd.indirect_dma_start(
                out=statg[:], out_offset=None, in_=t_stat[:],
                in_offset=bass.IndirectOffsetOnAxis(ap=nid_sb[:, c:c + 1], axis=0))
            xps = psw.tile([P, 132], dt.float32, space="PSUM", tag="GT",
                           bufs=1)
            for h, g in enumerate((memg, statg)):
                tp = ps.tile([P, P], dt.float32, space="PSUM", tag="tpx")
                nc.tensor.transpose(out=tp[:], in_=g[:], identity=ident[:])
                gt = sb3.tile([P, P], dt.float32, tag="gt")
                nc.vector.tensor_copy(out=gt[:], in_=tp[:])
                nc.tensor.matmul(out=xps[:, 0:MEM], lhsT=gt[:],
                                 rhs=encwT_sb[:, h, :],
                                 start=(h == 0), stop=(h == 1))
            nc.vector.tensor_tensor(out=x_sb[:, c, :], in0=xps[:, 0:MEM],
                                    in1=encb_sb[:], op=Alu.add)
            node_prep(c)

        # ---------- iterations ----------
        for it in range(ITERS):
            x_full = x_fulls[it]
            nc.gpsimd.collective_compute(
                "AllGather", mybir.AluOpType.bypass,
                replica_groups=[list(range(NCORES))],
                ins=[x_own.opt()], outs=[x_full.opt()])

            for g in range(NGRP):
                c0 = g * GN                    # first global chunk
                attrT_t = sb.tile([P, GN, P], dt.bfloat16, tag="attrT")
                nc.sync.dma_start(out=attrT_t[:],
                                  in_=attrT_r[:, c0:c0 + GN, :])
                attrN_t = sb.tile([P, GN, P], dt.bfloat16, tag="attrN")
                nc.sync.dma_start(out=attrN_t[:],
                                  in_=attrN_r[:, c0:c0 + GN, :])
                M_t = sb.tile([P, GN, P], dt.bfloat16, tag="Mt")
                nc.sync.dma_start(out=M_t[:], in_=Mt_r[:, c0:c0 + GN, :])
                xg = {}
                for which, K, tix, lim0, lim1, gk in (
                        (0, KL, t_xlo, 0, min(LO_LIMIT, NFULL), GL),
                        (1, KH, t_xhi, LO_LIMIT, NFULL, GH)):
                    if K == 0:
                        continue
                    kk0 = g * gk
                    kix = sb.tile([P, gk * 8], dt.int16, tag=f"kix{which}")
                    nc.sync.dma_start(out=kix[:],
                                      in_=tix[:, kk0 * 8:(kk0 + gk) * 8])
                    xt_ = sb.tile([P, gk, MEM], dt.bfloat16, tag=f"xg{which}")
                    for b0 in range(0, gk, MAXB):
                        b1 = min(b0 + MAXB, gk)
                        nc.gpsimd.dma_gather(
                            xt_[:, b0:b1, :], x_full[lim0:lim1, :],
                            kix[:, b0 * 8:b1 * 8],
                            (b1 - b0) * P, (b1 - b0) * P, MEM)
                    xg[which] = xt_

                hx = sb3.tile([P, GWIN, MEM], dt.float32, tag="hx", bufs=2)
                for wi in range(GWIN):
                    w = g * GWIN + wi          # global window index
                    GT = psw.tile([P, 132], dt.float32, space="PSUM",
                                  tag="GT", bufs=1)
                    HA = psw.tile([P, P], dt.float32, space="PSUM",
                                  tag="HA", bufs=1)
                    for k in range(NCH_W):
                        tc_ = wi * NCH_W + k   # chunk col within group tiles
                        if k < KL:
                            xt_, xc = xg[0], wi * KL + k
                        else:
                            xt_, xc = xg[1], wi * KH + (k - KL)
                        xrow = xt_[:, xc, :]
                        tpx = ps.tile([P, P], dt.bfloat16, space="PSUM",
                                      tag="tpx")
                        nc.tensor.transpose(out=tpx[:], in_=xrow,
                                            identity=ident_bf[:])
                        xgT = sb3.tile([P, P], dt.bfloat16, tag="xgT")
                        nc.vector.tensor_copy(out=xgT[:], in_=tpx[:])
                        ST = ps.tile([P, P], dt.float32, space="PSUM",
                                     tag="ST")
                        nc.tensor.matmul(out=ST[:], lhsT=xgT[:],
                                         rhs=qkT_sb[:, w, :],
                                         start=True, stop=False)
                        nc.tensor.matmul(out=ST[:], lhsT=attrT_t[:, tc_, :],
                                         rhs=qeT_sb[:, w, :],
                                         start=False, stop=True)
                        maskd = sb3.tile([P, P], dt.float32, tag="maskd")
                        nc.vector.tensor_tensor(out=maskd[:], in0=ST[:],
                                                in1=M_t[:, tc_, :], op=Alu.add)
                        wt = sb3.tile([P, P], dt.bfloat16, tag="wt")
                        nc.scalar.activation(out=wt[:], in_=maskd[:],
                                             func=Act.Exp)
                        nc.tensor.matmul(out=GT[:, 0:MEM], lhsT=xrow,
                                         rhs=wt[:],
                                         start=(k == 0), stop=False)
                        nc.tensor.matmul(out=HA[:], lhsT=attrN_t[:, tc_, :],
                                         rhs=wt[:],
                                         start=(k == 0), stop=(k == NCH_W - 1))
                        nc.tensor.matmul(out=GT[:, MEM:MEM + 1], lhsT=wt[:],
                                         rhs=ones_col[:],
                                         start=False, stop=(k == NCH_W - 1))

                    # --- window update ---
                    GT_sb = sb3.tile([P, P], dt.bfloat16, tag="GT_sb")
                    nc.vector.tensor_copy(out=GT_sb[:], in_=GT[:, 0:MEM])
                    HA_sb = sb3.tile([P, P], dt.bfloat16, tag="HA_sb")
                    nc.vector.tensor_copy(out=HA_sb[:], in_=HA[:])
                    Hf = ps.tile([P, P], dt.float32, space="PSUM", tag="ST")
                    nc.tensor.matmul(out=Hf[:], lhsT=GT_sb[:], rhs=wvT_sb[:],
                                     start=True, stop=False)
                    nc.tensor.matmul(out=Hf[:], lhsT=HA_sb[:], rhs=weT_sb[:],
                                     start=False, stop=True)
                    sden = sb3.tile([P, 1], dt.float32, tag="sden")
                    nc.vector.tensor_scalar(out=sden[:], in0=GT[:, MEM:MEM + 1],
                                            scalar1=1e-30, scalar2=None,
                                            op0=Alu.max)
                    nc.vector.reciprocal(out=sden[:], in_=sden[:])
                    nc.vector.scalar_tensor_tensor(
                        out=hx[:, wi, :], in0=Hf[:], scalar=sden[:, 0:1],
                        in1=xa_sb[:, w, :], op0=Alu.mult, op1=Alu.add)

                # --- group epilogue: tanh + x update + node prep ---
                th = sb3.tile([P, GWIN, MEM], dt.float32, tag="th")
                nc.scalar.activation(out=th[:], in_=hx[:], func=Act.Tanh)
                nc.vector.scalar_tensor_tensor(
                    out=x_sb[:, g * GWIN:(g + 1) * GWIN, :], in0=th[:],
                    scalar=EPS, in1=x_sb[:, g * GWIN:(g + 1) * GWIN, :],
                    op0=Alu.mult, op1=Alu.add)
                if it < ITERS - 1:
                    for wi in range(GWIN):
                        node_prep(g * GWIN + wi)

        nc.sync.dma_start(
            out=t_out.ap().rearrange("(c p) f -> p c f", p=P),
            in_=x_sb[:])

        for _pool in (dram, psw, ps, sb3, sb, perm):
            _pool.release()

    nc.compile()
    return nc


def kernel(n_id, edge_index, t, msg, static_node_features, memory, last_update,
           enc_w, enc_b, time_w, time_b, wq, bq, wk, bk, wv, bv, we, aW, abias):
    from concourse import bass_utils

    n_id = np.asarray(n_id)
    edge_index = np.asarray(edge_index)
    t = np.asarray(t)
    msg = np.asarray(msg, dtype=np.float32)
    num_nodes = memory.shape[0]
    f = np.float32

    cores, meta = _host_prep(n_id, edge_index, t, msg, last_update,
                             time_w, time_b)
    nc = _build(meta, num_nodes)

    isd = f(INV_SQRT_D)
    A_rhs = (np.asarray(aW, f).T - np.asarray(aW, f)
             - f(GAMMA) * np.eye(MEM, dtype=f))
    shared = {
        "memory": np.asarray(memory, dtype=f),
        "static_node_features": np.asarray(static_node_features, dtype=f),
        "enc_wT": np.ascontiguousarray(np.asarray(enc_w, f).T),
        "encb_rep": np.tile(np.asarray(enc_b, f).reshape(1, -1), (P, 1)),
        "wqT_isd": np.ascontiguousarray(np.asarray(wq, f).T * isd),
        "bq_isd": (np.asarray(bq, f) * isd).reshape(-1, 1),
        "wk": np.asarray(wk, f),
        "we": np.asarray(we, f),
        "wvT": np.ascontiguousarray(np.asarray(wv, f).T),
        "weT": np.ascontiguousarray(np.asarray(we, f).T),
        "A_rhs": np.ascontiguousarray(A_rhs),
        "abias2_rep": np.tile((np.asarray(abias, f)
                               + np.asarray(bv, f)).reshape(1, -1), (P, 1)),
    }
    in_maps = []
    for c in range(NCORES):
        m = dict(shared)
        for k in ("nid", "attrT", "attrN", "Mt", "xlo", "xhi"):
            m[k] = cores[c][k]
        in_maps.append(m)

    def unshard(results):
        N = meta["N"]
        local_of = meta["local_of"]
        bounds = meta["bounds"]
        out = np.zeros((N, MEM), dtype=f)
        for c in range(NCORES):
            nodes = np.arange(bounds[c], bounds[c + 1])
            out[nodes] = results[c]["out"][local_of[nodes]]
        return out

    if os.environ.get("KERNEL_SIM", "0") == "1":
        from concourse.bass_interp import MultiCoreSim
        sim = MultiCoreSim(nc, num_cores=NCORES, trace=False,
                           require_finite=False, require_nnan=False)
        cs = list(sim.cores.values())
        for ci, core in enumerate(cs):
            for k, v in in_maps[ci].items():
                core.tensor(k)[:] = v
        sim.simulate(check_with_hw=False, trace_hw=False)
        kernel.last_exec_time_ns = None
        return unshard([{"out": np.asarray(core.tensor("out"))} for core in cs])

    trace = os.environ.get("KERNEL_TRACE", "0") == "1"
    res = bass_utils.run_bass_kernel_spmd(
        nc, in_maps, core_ids=list(range(NCORES)), trace=trace)
    if trace:
        print("HW exec time:", res.exec_time_ns, "ns")
        kernel.last_exec_time_ns = res.exec_time_ns
        kernel.last_trace = res.instructions_and_trace
    return unshard(res.results)
